# revision 18
# baseline (speedup 1.0000x reference)
"""Trainium2 Bass kernel for a 3-layer GCN (PyG GCNConv x3 + global mean pool + linear).

Strategy (8 NeuronCores, SPMD), v2:
  - Nodes relabeled on the host (edge-balanced blocks of 128 via LPT) and
    partitioned contiguously across 8 cores; each core owns the edges whose
    destination lands in its partition.
  - All node-feature tables (x, h) are bf16: gathers move 256B rows, halving
    HBM traffic and SWDGE descriptor drain time.
  - The per-(block,group) scatter one-hot matrices (norm folded in) are
    precomputed on the host in bf16 and STREAMED from HBM via HWDGE — no
    on-device one-hot construction (the v1 VectorE bottleneck).
  - Self-loop diagonal matrices and mean-pool one-hots are layer-invariant
    bf16 constants resident in SBUF.
  - Per layer: SWDGE dma_gathers of 256B rows + scatter matmuls accumulating
    in PSUM (fp32); h' = relu(agg @ W + b) with bf16 weights; AllGather (bf16)
    shares node features; AllReduce at the mean pool; final linear on-device.
  - v3: layer 3 and the mean pool are linear, so they commute:
    pooled(agg3 @ W3 + b3) = pooled_agg3 @ W3 + b3, and pooled_agg3 regrouped
    BY SOURCE block needs no gather (h2 rows are core-local): it is one
    [128 x 64] matmul per block against a host-precomputed per-src-block
    weight matrix G (edge norms, self-loops and 1/graph-count folded in).
    This removes the third gather pass, its scatter-matrix stream, the second
    AllGather and the h2 store entirely.
"""

import numpy as np
import ml_dtypes

BF16 = ml_dtypes.bfloat16

# ---------------------------------------------------------------------------
# configuration
# ---------------------------------------------------------------------------

class CFG:
    def __init__(self, n_nodes, n_edges, n_graphs, n_cls, nblk, hi_base, lo_size,
                 chalf_lo=1024, chalf_hi=1024, super_blocks=4, nqueues=4):
        self.N = n_nodes
        self.E = n_edges
        self.NG = n_graphs
        self.NCLS = n_cls
        self.NBLK = nblk                       # blocks (of 128 dst nodes) per core
        self.NCORES = 8
        self.NQ = nqueues
        self.PER_CORE = nblk * 128
        self.NPAD = self.NCORES * self.PER_CORE
        self.LO_SIZE = lo_size                 # lo gather table = rows [0, LO_SIZE)
        self.HI_BASE = hi_base                 # hi gather table = rows [HI_BASE, NPAD)
        assert lo_size <= 32768 and (self.NPAD - hi_base) <= 32768
        self.CHALF_LO = chalf_lo               # lo edge slots per block
        self.CHALF_HI = chalf_hi
        assert chalf_lo % 128 == 0 and chalf_hi % 128 == 0
        self.NG_LO = chalf_lo // 128           # column-groups
        self.NG_HI = chalf_hi // 128
        self.NGRP = self.NG_LO + self.NG_HI    # edge groups per block
        self.CSLOT = chalf_lo + chalf_hi       # edge slots per block
        # super-blocks: group consecutive blocks into one gather call
        sb = []
        left = nblk
        while left > 0:
            take = min(super_blocks, left)
            sb.append(take)
            left -= take
        self.SB = sb
        self.SBMAX = max(sb)
        self.TCOL_LO = self.SBMAX * self.NG_LO     # tile cols of the lo tile
        self.TCOL_HI = self.SBMAX * self.NG_HI
        # scatter-matrix tensor: per block NGRP groups of 128 dst columns
        self.SCOLS = nblk * self.NGRP * 128
        self.STILE = self.SBMAX * self.NGRP * 128  # streamed slice per SB
        # eidx packing offsets (in int16 columns of the [128, *] index tile)
        self.eidx_off = []
        off = 0
        for nb in sb:
            lo_cols = nb * self.CHALF_LO // 16
            hi_cols = nb * self.CHALF_HI // 16
            self.eidx_off.append((off, off + lo_cols))
            off += lo_cols + hi_cols
        self.EIDX_COLS = off


FULL = CFG(n_nodes=50000, n_edges=800000, n_graphs=64, n_cls=16,
           nblk=49, hi_base=17408, lo_size=32768, super_blocks=7)


# ---------------------------------------------------------------------------
# host preprocessing
# ---------------------------------------------------------------------------

def _preprocess(cfg, x, edge_index, batch):
    """Relabel nodes, partition edges, build all device-side tables."""
    import heapq
    N = cfg.N
    src = np.asarray(edge_index[0], dtype=np.int64)
    dst = np.asarray(edge_index[1], dtype=np.int64)
    batch = np.asarray(batch, dtype=np.int64)

    deg = (np.bincount(dst, minlength=N) + 1).astype(np.float64)
    dinv = (1.0 / np.sqrt(deg)).astype(np.float32)
    dinv2 = (dinv * dinv).astype(np.float32)   # self-loop weight

    norm_all = (dinv[src] * dinv[dst]).astype(np.float32)

    # --- LPT assignment of nodes to blocks, balancing incoming edge counts ---
    w = np.bincount(dst, minlength=N)          # edge slots demanded per node
    nblocks = cfg.NCORES * cfg.NBLK
    order = np.argsort(-w, kind="stable")
    heap = [(0, 0, b) for b in range(nblocks)]
    heapq.heapify(heap)
    blk_of = np.empty(N, np.int64)
    pos_of = np.empty(N, np.int64)
    for v in order:
        while True:
            load, cnt, b = heapq.heappop(heap)
            if cnt < 128:
                break
        blk_of[v] = b
        pos_of[v] = cnt
        heapq.heappush(heap, (load + int(w[v]), cnt + 1, b))
    new_id = blk_of * 128 + pos_of                 # [N] -> [0, NPAD)

    x_perm = np.zeros((cfg.NPAD, x.shape[1]), dtype=np.float32)
    x_perm[new_id] = np.asarray(x, dtype=np.float32)
    x_perm = x_perm.astype(BF16)

    nsrc = new_id[src]
    ndst = new_id[dst]
    nb_of_e = ndst // 128
    eorder = np.argsort(nb_of_e, kind="stable")
    nsrc, ndst, norm_all, nb_of_e = (nsrc[eorder], ndst[eorder],
                                     norm_all[eorder], nb_of_e[eorder])
    blk_start = np.searchsorted(nb_of_e, np.arange(nblocks + 1))

    dinv2_new = np.zeros(cfg.NPAD, dtype=np.float32)
    dinv2_new[new_id] = dinv2

    # fused layer-3 + mean-pool weights, grouped by SOURCE node:
    # pooled_agg3[g] = sum_e (dinv_s*dinv_d/cnt_gd) h2[src_e] + sum_v dinv2_v/cnt_gv h2[v]
    counts = np.bincount(batch, minlength=cfg.NG).astype(np.float32)
    cnt = np.maximum(counts, 1.0)
    alpha = dinv[src] * dinv[dst] / cnt[batch[dst]]
    gall = np.bincount(new_id[src] * cfg.NG + batch[dst], weights=alpha,
                       minlength=cfg.NPAD * cfg.NG)
    gall += np.bincount(new_id * cfg.NG + batch, weights=dinv2 / cnt[batch],
                        minlength=cfg.NPAD * cfg.NG)
    gall = gall.reshape(cfg.NPAD, cfg.NG).astype(np.float32)

    pidx = np.arange(128)
    eidx_all, smat_all, diag_all, po_all = [], [], [], []
    for c in range(cfg.NCORES):
        eidx = np.zeros((16, cfg.EIDX_COLS), dtype=np.int16)
        smat = np.zeros((128, cfg.SCOLS), dtype=np.float32)
        for s, nbk in enumerate(cfg.SB):
            lo_idx_cat, hi_idx_cat = [], []
            for j in range(nbk):
                blk_local = s_block_base(cfg, s) + j
                b = c * cfg.NBLK + blk_local
                lo_e, hi_e = _split_block_edges(
                    cfg, nsrc[blk_start[b]:blk_start[b + 1]],
                    ndst[blk_start[b]:blk_start[b + 1]],
                    norm_all[blk_start[b]:blk_start[b + 1]])
                (li, ld, ln), (hi_, hd, hn) = lo_e, hi_e
                lo_idx_cat.append(li)
                hi_idx_cat.append(hi_)
                # scatter matrices with norm folded in (bf16 later)
                for goff, dvals, nvals in ((0, ld, ln), (cfg.NG_LO, hd, hn)):
                    k = np.flatnonzero(nvals != 0)
                    part = k % 128
                    grp = k // 128 + goff
                    col = (blk_local * cfg.NGRP + grp) * 128 + dvals[k].astype(np.int64)
                    smat[part, col] = nvals[k]
            lo_cat = np.concatenate(lo_idx_cat)
            hi_cat = np.concatenate(hi_idx_cat)
            o0, o1 = cfg.eidx_off[s]
            eidx[:, o0:o1] = lo_cat.reshape(-1, 16).T
            eidx[:, o1:o1 + hi_cat.size // 16] = hi_cat.reshape(-1, 16).T
        base = c * cfg.PER_CORE
        # self-loop diagonal matrices [128, NBLK*128]
        diag = np.zeros((128, cfg.NBLK * 128), dtype=np.float32)
        dv = dinv2_new[base:base + cfg.PER_CORE].reshape(cfg.NBLK, 128)
        for blk in range(cfg.NBLK):
            diag[pidx, blk * 128 + pidx] = dv[blk]
        # fused layer3+pool weights [128, NBLK*NG] (src-block-major)
        po = (gall[base:base + cfg.PER_CORE]
              .reshape(cfg.NBLK, 128, cfg.NG)
              .transpose(1, 0, 2)
              .reshape(128, cfg.NBLK * cfg.NG))
        eidx_all.append(np.tile(eidx, (8, 1)))
        smat_all.append(smat.astype(BF16))
        diag_all.append(diag.astype(BF16))
        po_all.append(po.astype(BF16).copy())

    return dict(x_perm=x_perm, eidx=eidx_all, smat=smat_all, diag=diag_all,
                po=po_all)


def s_block_base(cfg, s):
    return sum(cfg.SB[:s])


def _split_block_edges(cfg, nsrc_b, ndst_b, nrm_b):
    """Split one block's edges into lo/hi gather halves and pad."""
    CH_LO, CH_HI = cfg.CHALF_LO, cfg.CHALF_HI
    n = nsrc_b.size
    assert n <= CH_LO + CH_HI, f"block overflow: {n} > {CH_LO + CH_HI}"
    strict_lo = nsrc_b < cfg.HI_BASE
    strict_hi = nsrc_b >= cfg.LO_SIZE
    mid = ~strict_lo & ~strict_hi
    n_strict_lo = int(strict_lo.sum())
    n_strict_hi = int(strict_hi.sum())
    assert n_strict_lo <= CH_LO, "lo half overflow"
    assert n_strict_hi <= CH_HI, "hi half overflow"
    take_mid_lo = min(int(mid.sum()), CH_LO - n_strict_lo)
    mid_idx = np.flatnonzero(mid)
    lo_sel = np.concatenate([np.flatnonzero(strict_lo), mid_idx[:take_mid_lo]])
    hi_sel = np.concatenate([np.flatnonzero(strict_hi), mid_idx[take_mid_lo:]])
    assert hi_sel.size <= CH_HI, "hi half overflow after balance"
    dl = (ndst_b % 128).astype(np.int64)

    def pack(sel, base, cap):
        idx = np.zeros(cap, np.int16)
        d = np.zeros(cap, np.int64)
        nm = np.zeros(cap, np.float32)
        k = sel.size
        idx[:k] = (nsrc_b[sel] - base).astype(np.int16)
        d[:k] = dl[sel]
        nm[:k] = nrm_b[sel]
        return idx, d, nm

    return pack(lo_sel, 0, CH_LO), pack(hi_sel, cfg.HI_BASE, CH_HI)


# ---------------------------------------------------------------------------
# device program
# ---------------------------------------------------------------------------

_PROGRAM_CACHE = {}


def _build_program(cfg):
    import concourse.bacc as bacc
    import concourse.tile as tile
    import concourse.mybir as mybir

    f32 = mybir.dt.float32
    bf16 = mybir.dt.bfloat16
    i16 = mybir.dt.int16
    AF = mybir.ActivationFunctionType
    OP = mybir.AluOpType

    nc = bacc.Bacc("TRN2", target_bir_lowering=False, debug=False,
                   num_devices=cfg.NCORES, num_swdge_queues=cfg.NQ,
                   dynamic_dma_scratch_size=24576)

    D = 128
    xp = nc.dram_tensor("xp", [cfg.NPAD, D], bf16, kind="ExternalInput")
    xself = nc.dram_tensor("xself", [cfg.PER_CORE, D], bf16, kind="ExternalInput")
    eidx = nc.dram_tensor("eidx", [128, cfg.EIDX_COLS], i16, kind="ExternalInput")
    smat = nc.dram_tensor("smat", [128, cfg.SCOLS], bf16, kind="ExternalInput")
    diag = nc.dram_tensor("diag", [128, cfg.NBLK * 128], bf16, kind="ExternalInput")
    po = nc.dram_tensor("po", [128, cfg.NBLK * cfg.NG], bf16, kind="ExternalInput")
    wts = [nc.dram_tensor(f"w{l}", [D, D], bf16, kind="ExternalInput") for l in range(2)]
    w3 = nc.dram_tensor("w3", [D, D], f32, kind="ExternalInput")
    bbs = [nc.dram_tensor(f"bb{l}", [128, D], f32, kind="ExternalInput") for l in range(2)]
    b3c = nc.dram_tensor("b3c", [128, 1], f32, kind="ExternalInput")
    linw = nc.dram_tensor("linw", [D, cfg.NCLS], f32, kind="ExternalInput")
    linb = nc.dram_tensor("linb", [cfg.NCLS, 1], f32, kind="ExternalInput")

    out_t = nc.dram_tensor("out_t", [cfg.NCLS, cfg.NG], f32, kind="ExternalOutput")

    h_loc = [nc.dram_tensor("h_loc0", [cfg.PER_CORE, D], bf16, kind="Internal")]
    h_ful = [nc.dram_tensor("h_ful0", [cfg.NPAD, D], bf16, kind="Internal",
                            addr_space="Shared")]
    pool_in = nc.dram_tensor("pool_in", [128, cfg.NG], f32, kind="Internal")
    pool_out = nc.dram_tensor("pool_out", [128, cfg.NG], f32, kind="Internal",
                              addr_space="Shared")

    groups = [list(range(cfg.NCORES))]
    qctr = [0]

    def next_q():
        q = qctr[0] % cfg.NQ
        qctr[0] += 1
        return q

    with tile.TileContext(nc) as tc:
        with tc.tile_pool(name="const", bufs=1) as cp, \
             tc.tile_pool(name="glo", bufs=3) as gplo, \
             tc.tile_pool(name="ghi", bufs=3) as gphi, \
             tc.tile_pool(name="smat", bufs=2) as sp_, \
             tc.tile_pool(name="sbwork", bufs=3) as wp, \
             tc.tile_pool(name="psum_a", bufs=2, space="PSUM") as pa, \
             tc.tile_pool(name="psum_h", bufs=2, space="PSUM") as ph, \
             tc.tile_pool(name="psum_p", bufs=1, space="PSUM") as pp:

            def load_const(t, shape, dtype=f32):
                tl = cp.tile(list(shape), dtype, tag=t.name)
                nc.sync.dma_start(out=tl[:], in_=t.ap())
                return tl

            eidx_t = load_const(eidx, [128, cfg.EIDX_COLS], i16)
            diag_t = load_const(diag, [128, cfg.NBLK * 128], bf16)
            po_t = load_const(po, [128, cfg.NBLK * cfg.NG], bf16)
            wts_t = [load_const(w, [D, D], bf16) for w in wts]
            w3_t = load_const(w3, [D, D])
            bbs_t = [load_const(b, [128, D]) for b in bbs]
            b3c_t = load_const(b3c, [128, 1])
            linw_t = load_const(linw, [D, cfg.NCLS])
            linb_t = load_const(linb, [cfg.NCLS, 1])

            pool_ps = pp.tile([128, cfg.NG], f32)

            def table_views(l):
                table = xp if l == 0 else h_ful[l - 1]
                return (table.ap()[0:cfg.LO_SIZE, :],
                        table.ap()[cfg.HI_BASE:cfg.NPAD, :])

            def issue_gathers(l, s, nbk, prep_sems=None):
                lo_view, hi_view = table_views(l)
                tglo = gplo.tile([128, cfg.TCOL_LO, 128], bf16, tag="tglo")
                tghi = gphi.tile([128, cfg.TCOL_HI, 128], bf16, tag="tghi")
                o0, o1 = cfg.eidx_off[s]
                n_lo = nbk * cfg.CHALF_LO
                n_hi = nbk * cfg.CHALF_HI
                q0, q1 = next_q(), next_q()
                kw0 = dict(prepare_only=True, sem=prep_sems[0]) if prep_sems else {}
                kw1 = dict(prepare_only=True, sem=prep_sems[1]) if prep_sems else {}
                nc.gpsimd.dma_gather(
                    tglo[:, 0:nbk * cfg.NG_LO, :], lo_view,
                    eidx_t[:, o0:o0 + n_lo // 16],
                    num_idxs=n_lo, num_idxs_reg=n_lo, elem_size=128,
                    single_packet=False, queue_num=q0, **kw0)
                nc.gpsimd.dma_gather(
                    tghi[:, 0:nbk * cfg.NG_HI, :], hi_view,
                    eidx_t[:, o1:o1 + n_hi // 16],
                    num_idxs=n_hi, num_idxs_reg=n_hi, elem_size=128,
                    single_packet=False, queue_num=q1, **kw1)
                return tglo, tghi, (q0, q1)

            N_PREP = 0
            pre_tiles = {}
            prep_queues = []

            for l in range(2):
                selftab = xself if l == 0 else h_loc[l - 1]
                for s, nbk in enumerate(cfg.SB):
                    bbase = s_block_base(cfg, s)
                    if (l, s) in pre_tiles:
                        tglo, tghi = pre_tiles.pop((l, s))
                    else:
                        tglo, tghi, _ = issue_gathers(l, s, nbk)
                    smat_t = sp_.tile([128, cfg.STILE], bf16, tag="smat_t")
                    nc.sync.dma_start(
                        out=smat_t[:, 0:nbk * cfg.NGRP * 128],
                        in_=smat.ap()[:, bbase * cfg.NGRP * 128:
                                      (bbase + nbk) * cfg.NGRP * 128])
                    for j in range(nbk):
                        blk = bbase + j
                        hb = wp.tile([128, 128], bf16, tag="hb")
                        aggT = pa.tile([128, 128], f32, tag="aggT")
                        for gg in range(cfg.NGRP):
                            if gg < cfg.NG_LO:
                                tcol = j * cfg.NG_LO + gg
                                src_sl = tglo[:, tcol, :]
                            else:
                                tcol = j * cfg.NG_HI + (gg - cfg.NG_LO)
                                src_sl = tghi[:, tcol, :]
                            scol = (j * cfg.NGRP + gg) * 128
                            nc.tensor.matmul(
                                aggT[:], lhsT=src_sl,
                                rhs=smat_t[:, scol:scol + 128],
                                start=(gg == 0), stop=False)
                        # self-loop: contiguous row read + diagonal matrix
                        nc.sync.dma_start(
                            out=hb[:],
                            in_=selftab.ap()[blk * 128:(blk + 1) * 128, :])
                        nc.tensor.matmul(
                            aggT[:], lhsT=hb[:],
                            rhs=diag_t[:, blk * 128:(blk + 1) * 128],
                            start=False, stop=True)
                        aggs = wp.tile([128, 128], bf16, tag="aggs")
                        nc.scalar.activation(aggs[:], aggT[:], AF.Copy)
                        hp = ph.tile([128, 128], f32, tag="hp")
                        nc.tensor.matmul(hp[:], lhsT=aggs[:], rhs=wts_t[l][:],
                                         start=True, stop=True)
                        hs = wp.tile([128, 128], bf16, tag="hs")
                        nc.vector.tensor_tensor(hs[:], hp[:], bbs_t[l][:],
                                                OP.add)
                        nc.vector.tensor_relu(hs[:], hs[:])
                        if l == 0:
                            nc.sync.dma_start(
                                out=h_loc[0].ap()[blk * 128:(blk + 1) * 128, :],
                                in_=hs[:])
                        else:
                            # fused layer3 aggregation + mean pool, by source
                            nc.tensor.matmul(
                                pool_ps[:], lhsT=hs[:],
                                rhs=po_t[:, blk * cfg.NG:(blk + 1) * cfg.NG],
                                start=(blk == 0), stop=(blk == cfg.NBLK - 1))
                if l == 0:
                    # prepare layer-2's first gathers now: descriptor
                    # generation (the serial Q7 bottleneck) overlaps the
                    # tail compute and the AllGather; triggers fire after.
                    for ps in range(N_PREP):
                        sems = (nc.alloc_semaphore(f"pgl{ps}"),
                                nc.alloc_semaphore(f"pgh{ps}"))
                        tg, th, qs = issue_gathers(1, ps, cfg.SB[ps],
                                                   prep_sems=sems)
                        pre_tiles[(1, ps)] = (tg, th)
                        prep_queues.extend(qs)
                    nc.gpsimd.collective_compute(
                        "AllGather", mybir.AluOpType.bypass,
                        replica_groups=groups,
                        ins=[h_loc[0].ap()], outs=[h_ful[0].ap()])
                    if prep_queues:
                        # Pool-engine read of h_ful0: Tile attaches the
                        # AllGather-completion wait here; the triggers that
                        # follow in Pool program order are therefore safe.
                        guard = wp.tile([1, 64], bf16, tag="agguard")
                        nc.gpsimd.dma_start(out=guard[:],
                                            in_=h_ful[0].ap()[0:1, 0:64])
                    for q in prep_queues:
                        nc.gpsimd.trigger_dma(count=None, queue_num=q)

            # epilogue: AllReduce pooled_agg3, then (pooled_agg3 @ W3 + b3) @ lin
            pool_sb = wp.tile([128, cfg.NG], f32, tag="pool_sb")
            nc.scalar.activation(pool_sb[:], pool_ps[:], AF.Copy)
            nc.sync.dma_start(out=pool_in.ap(), in_=pool_sb[:])
            nc.gpsimd.collective_compute(
                "AllReduce", mybir.AluOpType.add, replica_groups=groups,
                ins=[pool_in.ap()], outs=[pool_out.ap()])
            psum2 = wp.tile([128, cfg.NG], f32, tag="psum2")
            nc.sync.dma_start(out=psum2[:], in_=pool_out.ap())
            h3_ps = pp.tile([128, cfg.NG], f32, tag="h3_ps")
            nc.tensor.matmul(h3_ps[:], lhsT=w3_t[:], rhs=psum2[:],
                             start=True, stop=True)
            h3_sb = wp.tile([128, cfg.NG], f32, tag="h3_sb")
            nc.vector.tensor_scalar(h3_sb[:], h3_ps[:], b3c_t[:, 0:1], None,
                                    OP.add)
            out_ps = pp.tile([cfg.NCLS, cfg.NG], f32, tag="out_ps")
            nc.tensor.matmul(out_ps[:], lhsT=linw_t[:], rhs=h3_sb[:],
                             start=True, stop=True)
            outs = wp.tile([cfg.NCLS, cfg.NG], f32, tag="outs")
            nc.vector.tensor_scalar(outs[:], out_ps[:], linb_t[:, 0:1], None,
                                    OP.add)
            nc.sync.dma_start(out=out_t.ap(), in_=outs[:])

    nc.compile()
    return nc


def _get_program(cfg):
    key = (cfg.N, cfg.E, cfg.NG, cfg.NCLS, cfg.NBLK, cfg.CSLOT, cfg.NQ)
    if key not in _PROGRAM_CACHE:
        _PROGRAM_CACHE[key] = _build_program(cfg)
    return _PROGRAM_CACHE[key]


# ---------------------------------------------------------------------------
# entry point
# ---------------------------------------------------------------------------

def _run(cfg, x, edge_index, batch, W1, b1, W2, b2, W3, b3, lin_w, lin_b,
         trace=False):
    from concourse import bass_utils

    pre = _preprocess(cfg, x, edge_index, batch)
    nc = _get_program(cfg)

    shared = {
        "w0": np.asarray(W1, np.float32).astype(BF16),
        "w1": np.asarray(W2, np.float32).astype(BF16),
        "w3": np.asarray(W3, np.float32),
        "bb0": np.tile(np.asarray(b1, np.float32), (128, 1)),
        "bb1": np.tile(np.asarray(b2, np.float32), (128, 1)),
        "b3c": np.asarray(b3, np.float32).reshape(128, 1),
        "linw": np.asarray(lin_w, np.float32),
        "linb": np.asarray(lin_b, np.float32).reshape(cfg.NCLS, 1),
    }
    in_maps = []
    for c in range(cfg.NCORES):
        m = dict(shared)
        m["xp"] = pre["x_perm"]
        m["xself"] = pre["x_perm"][c * cfg.PER_CORE:(c + 1) * cfg.PER_CORE]
        m["eidx"] = pre["eidx"][c]
        m["smat"] = pre["smat"][c]
        m["diag"] = pre["diag"][c]
        m["po"] = pre["po"][c]
        in_maps.append(m)

    res = bass_utils.run_bass_kernel_spmd(
        nc, in_maps, core_ids=list(range(cfg.NCORES)), trace=trace)
    out = np.asarray(res.results[0]["out_t"]).T.copy()
    return out, res


def kernel(x, edge_index, batch, W1, b1, W2, b2, W3, b3, lin_w, lin_b):
    out, _ = _run(FULL, x, edge_index, batch, W1, b1, W2, b2, W3, b3,
                  lin_w, lin_b, trace=False)
    return out


# revision 21
# speedup vs baseline: 1.1763x; 1.1763x over previous
"""Trainium2 Bass kernel for a 3-layer GCN (PyG GCNConv x3 + global mean pool + linear).

Strategy (8 NeuronCores, SPMD), v2:
  - Nodes relabeled on the host (edge-balanced blocks of 128 via LPT) and
    partitioned contiguously across 8 cores; each core owns the edges whose
    destination lands in its partition.
  - All node-feature tables (x, h) are bf16: gathers move 256B rows, halving
    HBM traffic and SWDGE descriptor drain time.
  - The per-(block,group) scatter one-hot matrices (norm folded in) are
    precomputed on the host in bf16 and STREAMED from HBM via HWDGE — no
    on-device one-hot construction (the v1 VectorE bottleneck).
  - Self-loop diagonal matrices and mean-pool one-hots are layer-invariant
    bf16 constants resident in SBUF.
  - Per layer: SWDGE dma_gathers of 256B rows + scatter matmuls accumulating
    in PSUM (fp32); h' = relu(agg @ W + b) with bf16 weights; AllGather (bf16)
    shares node features; AllReduce at the mean pool; final linear on-device.
  - v3: layer 3 and the mean pool are linear, so they commute:
    pooled(agg3 @ W3 + b3) = pooled_agg3 @ W3 + b3, and pooled_agg3 regrouped
    BY SOURCE block needs no gather (h2 rows are core-local): it is one
    [128 x 64] matmul per block against a host-precomputed per-src-block
    weight matrix G (edge norms, self-loops and 1/graph-count folded in).
    This removes the third gather pass, its scatter-matrix stream, the second
    AllGather and the h2 store entirely.
"""

import numpy as np
import ml_dtypes

BF16 = ml_dtypes.bfloat16

# ---------------------------------------------------------------------------
# configuration
# ---------------------------------------------------------------------------

class CFG:
    def __init__(self, n_nodes, n_edges, n_graphs, n_cls, nblk, hi_base, lo_size,
                 chalf_lo=1024, chalf_hi=1024, super_blocks=4, nqueues=4):
        self.N = n_nodes
        self.E = n_edges
        self.NG = n_graphs
        self.NCLS = n_cls
        self.NBLK = nblk                       # blocks (of 128 dst nodes) per core
        self.NCORES = 8
        self.NQ = nqueues
        self.PER_CORE = nblk * 128
        self.NPAD = self.NCORES * self.PER_CORE
        self.LO_SIZE = lo_size                 # lo gather table = rows [0, LO_SIZE)
        self.HI_BASE = hi_base                 # hi gather table = rows [HI_BASE, NPAD)
        assert lo_size <= 32768 and (self.NPAD - hi_base) <= 32768
        self.CHALF_LO = chalf_lo               # lo edge slots per block
        self.CHALF_HI = chalf_hi
        assert chalf_lo % 128 == 0 and chalf_hi % 128 == 0
        self.NG_LO = chalf_lo // 128           # column-groups
        self.NG_HI = chalf_hi // 128
        self.NGRP = self.NG_LO + self.NG_HI    # edge groups per block
        self.CSLOT = chalf_lo + chalf_hi       # edge slots per block
        # super-blocks: group consecutive blocks into one gather call
        sb = []
        left = nblk
        while left > 0:
            take = min(super_blocks, left)
            sb.append(take)
            left -= take
        self.SB = sb
        self.SBMAX = max(sb)
        self.TCOL_LO = self.SBMAX * self.NG_LO     # tile cols of the lo tile
        self.TCOL_HI = self.SBMAX * self.NG_HI
        # scatter-matrix tensor: per block NGRP groups of 128 dst columns
        self.SCOLS = nblk * self.NGRP * 128
        self.STILE = self.SBMAX * self.NGRP * 128  # streamed slice per SB
        # eidx packing offsets (in int16 columns of the [128, *] index tile)
        self.eidx_off = []
        off = 0
        for nb in sb:
            lo_cols = nb * self.CHALF_LO // 16
            hi_cols = nb * self.CHALF_HI // 16
            self.eidx_off.append((off, off + lo_cols))
            off += lo_cols + hi_cols
        self.EIDX_COLS = off


FULL = CFG(n_nodes=50000, n_edges=800000, n_graphs=64, n_cls=16,
           nblk=49, hi_base=17408, lo_size=32768)


# ---------------------------------------------------------------------------
# host preprocessing
# ---------------------------------------------------------------------------

def _preprocess(cfg, x, edge_index, batch):
    """Relabel nodes, partition edges, build all device-side tables."""
    import heapq
    N = cfg.N
    src = np.asarray(edge_index[0], dtype=np.int64)
    dst = np.asarray(edge_index[1], dtype=np.int64)
    batch = np.asarray(batch, dtype=np.int64)

    deg = (np.bincount(dst, minlength=N) + 1).astype(np.float64)
    dinv = (1.0 / np.sqrt(deg)).astype(np.float32)
    dinv2 = (dinv * dinv).astype(np.float32)   # self-loop weight

    norm_all = (dinv[src] * dinv[dst]).astype(np.float32)

    # --- LPT assignment of nodes to blocks, balancing incoming edge counts ---
    w = np.bincount(dst, minlength=N)          # edge slots demanded per node
    nblocks = cfg.NCORES * cfg.NBLK
    order = np.argsort(-w, kind="stable")
    heap = [(0, 0, b) for b in range(nblocks)]
    heapq.heapify(heap)
    blk_of = np.empty(N, np.int64)
    pos_of = np.empty(N, np.int64)
    for v in order:
        while True:
            load, cnt, b = heapq.heappop(heap)
            if cnt < 128:
                break
        blk_of[v] = b
        pos_of[v] = cnt
        heapq.heappush(heap, (load + int(w[v]), cnt + 1, b))
    new_id = blk_of * 128 + pos_of                 # [N] -> [0, NPAD)

    x_perm = np.zeros((cfg.NPAD, x.shape[1]), dtype=np.float32)
    x_perm[new_id] = np.asarray(x, dtype=np.float32)
    x_perm = x_perm.astype(BF16)

    nsrc = new_id[src]
    ndst = new_id[dst]
    nb_of_e = ndst // 128
    eorder = np.argsort(nb_of_e, kind="stable")
    nsrc, ndst, norm_all, nb_of_e = (nsrc[eorder], ndst[eorder],
                                     norm_all[eorder], nb_of_e[eorder])
    blk_start = np.searchsorted(nb_of_e, np.arange(nblocks + 1))

    dinv2_new = np.zeros(cfg.NPAD, dtype=np.float32)
    dinv2_new[new_id] = dinv2

    # fused layer-3 + mean-pool weights, grouped by SOURCE node:
    # pooled_agg3[g] = sum_e (dinv_s*dinv_d/cnt_gd) h2[src_e] + sum_v dinv2_v/cnt_gv h2[v]
    counts = np.bincount(batch, minlength=cfg.NG).astype(np.float32)
    cnt = np.maximum(counts, 1.0)
    alpha = dinv[src] * dinv[dst] / cnt[batch[dst]]
    gall = np.bincount(new_id[src] * cfg.NG + batch[dst], weights=alpha,
                       minlength=cfg.NPAD * cfg.NG)
    gall += np.bincount(new_id * cfg.NG + batch, weights=dinv2 / cnt[batch],
                        minlength=cfg.NPAD * cfg.NG)
    gall = gall.reshape(cfg.NPAD, cfg.NG).astype(np.float32)

    pidx = np.arange(128)
    eidx_all, smat_all, diag_all, po_all = [], [], [], []
    for c in range(cfg.NCORES):
        eidx = np.zeros((16, cfg.EIDX_COLS), dtype=np.int16)
        smat = np.zeros((128, cfg.SCOLS), dtype=np.float32)
        for s, nbk in enumerate(cfg.SB):
            lo_idx_cat, hi_idx_cat = [], []
            for j in range(nbk):
                blk_local = s_block_base(cfg, s) + j
                b = c * cfg.NBLK + blk_local
                lo_e, hi_e = _split_block_edges(
                    cfg, nsrc[blk_start[b]:blk_start[b + 1]],
                    ndst[blk_start[b]:blk_start[b + 1]],
                    norm_all[blk_start[b]:blk_start[b + 1]])
                (li, ld, ln), (hi_, hd, hn) = lo_e, hi_e
                lo_idx_cat.append(li)
                hi_idx_cat.append(hi_)
                # scatter matrices with norm folded in (bf16 later)
                for goff, dvals, nvals in ((0, ld, ln), (cfg.NG_LO, hd, hn)):
                    k = np.flatnonzero(nvals != 0)
                    part = k % 128
                    grp = k // 128 + goff
                    col = (blk_local * cfg.NGRP + grp) * 128 + dvals[k].astype(np.int64)
                    smat[part, col] = nvals[k]
            lo_cat = np.concatenate(lo_idx_cat)
            hi_cat = np.concatenate(hi_idx_cat)
            o0, o1 = cfg.eidx_off[s]
            eidx[:, o0:o1] = lo_cat.reshape(-1, 16).T
            eidx[:, o1:o1 + hi_cat.size // 16] = hi_cat.reshape(-1, 16).T
        base = c * cfg.PER_CORE
        # self-loop diagonal matrices [128, NBLK*128]
        diag = np.zeros((128, cfg.NBLK * 128), dtype=np.float32)
        dv = dinv2_new[base:base + cfg.PER_CORE].reshape(cfg.NBLK, 128)
        for blk in range(cfg.NBLK):
            diag[pidx, blk * 128 + pidx] = dv[blk]
        # fused layer3+pool weights [128, NBLK*NG] (src-block-major)
        po = (gall[base:base + cfg.PER_CORE]
              .reshape(cfg.NBLK, 128, cfg.NG)
              .transpose(1, 0, 2)
              .reshape(128, cfg.NBLK * cfg.NG))
        eidx_all.append(np.tile(eidx, (8, 1)))
        smat_all.append(smat.astype(BF16))
        diag_all.append(diag.astype(BF16))
        po_all.append(po.astype(BF16).copy())

    return dict(x_perm=x_perm, eidx=eidx_all, smat=smat_all, diag=diag_all,
                po=po_all)


def s_block_base(cfg, s):
    return sum(cfg.SB[:s])


def _split_block_edges(cfg, nsrc_b, ndst_b, nrm_b):
    """Split one block's edges into lo/hi gather halves and pad."""
    CH_LO, CH_HI = cfg.CHALF_LO, cfg.CHALF_HI
    n = nsrc_b.size
    assert n <= CH_LO + CH_HI, f"block overflow: {n} > {CH_LO + CH_HI}"
    strict_lo = nsrc_b < cfg.HI_BASE
    strict_hi = nsrc_b >= cfg.LO_SIZE
    mid = ~strict_lo & ~strict_hi
    n_strict_lo = int(strict_lo.sum())
    n_strict_hi = int(strict_hi.sum())
    assert n_strict_lo <= CH_LO, "lo half overflow"
    assert n_strict_hi <= CH_HI, "hi half overflow"
    take_mid_lo = min(int(mid.sum()), CH_LO - n_strict_lo)
    mid_idx = np.flatnonzero(mid)
    lo_sel = np.concatenate([np.flatnonzero(strict_lo), mid_idx[:take_mid_lo]])
    hi_sel = np.concatenate([np.flatnonzero(strict_hi), mid_idx[take_mid_lo:]])
    lo_sel = lo_sel[np.argsort(nsrc_b[lo_sel], kind="stable")]
    hi_sel = hi_sel[np.argsort(nsrc_b[hi_sel], kind="stable")]
    assert hi_sel.size <= CH_HI, "hi half overflow after balance"
    dl = (ndst_b % 128).astype(np.int64)

    def pack(sel, base, cap):
        idx = np.zeros(cap, np.int16)
        d = np.zeros(cap, np.int64)
        nm = np.zeros(cap, np.float32)
        k = sel.size
        idx[:k] = (nsrc_b[sel] - base).astype(np.int16)
        d[:k] = dl[sel]
        nm[:k] = nrm_b[sel]
        return idx, d, nm

    return pack(lo_sel, 0, CH_LO), pack(hi_sel, cfg.HI_BASE, CH_HI)


# ---------------------------------------------------------------------------
# device program
# ---------------------------------------------------------------------------

_PROGRAM_CACHE = {}


def _build_program(cfg):
    import concourse.bacc as bacc
    import concourse.tile as tile
    import concourse.mybir as mybir

    f32 = mybir.dt.float32
    bf16 = mybir.dt.bfloat16
    i16 = mybir.dt.int16
    AF = mybir.ActivationFunctionType
    OP = mybir.AluOpType

    nc = bacc.Bacc("TRN2", target_bir_lowering=False, debug=False,
                   num_devices=cfg.NCORES, num_swdge_queues=cfg.NQ,
                   dynamic_dma_scratch_size=32768)

    D = 128
    xp = nc.dram_tensor("xp", [cfg.NPAD, D], bf16, kind="ExternalInput")
    xself = nc.dram_tensor("xself", [cfg.PER_CORE, D], bf16, kind="ExternalInput")
    eidx = nc.dram_tensor("eidx", [128, cfg.EIDX_COLS], i16, kind="ExternalInput")
    smat = nc.dram_tensor("smat", [128, cfg.SCOLS], bf16, kind="ExternalInput")
    diag = nc.dram_tensor("diag", [128, cfg.NBLK * 128], bf16, kind="ExternalInput")
    po = nc.dram_tensor("po", [128, cfg.NBLK * cfg.NG], bf16, kind="ExternalInput")
    wts = [nc.dram_tensor(f"w{l}", [D, D], bf16, kind="ExternalInput") for l in range(2)]
    w3 = nc.dram_tensor("w3", [D, D], f32, kind="ExternalInput")
    bbs = [nc.dram_tensor(f"bb{l}", [128, D], f32, kind="ExternalInput") for l in range(2)]
    b3c = nc.dram_tensor("b3c", [128, 1], f32, kind="ExternalInput")
    linw = nc.dram_tensor("linw", [D, cfg.NCLS], f32, kind="ExternalInput")
    linb = nc.dram_tensor("linb", [cfg.NCLS, 1], f32, kind="ExternalInput")

    out_t = nc.dram_tensor("out_t", [cfg.NCLS, cfg.NG], f32, kind="ExternalOutput")

    h_loc = [nc.dram_tensor("h_loc0", [cfg.PER_CORE, D], bf16, kind="Internal")]
    h_ful = [nc.dram_tensor("h_ful0", [cfg.NPAD, D], bf16, kind="Internal",
                            addr_space="Shared")]
    pool_in = nc.dram_tensor("pool_in", [128, cfg.NG], f32, kind="Internal")
    pool_out = nc.dram_tensor("pool_out", [128, cfg.NG], f32, kind="Internal",
                              addr_space="Shared")

    groups = [list(range(cfg.NCORES))]
    qctr = [0]

    def next_q():
        q = qctr[0] % cfg.NQ
        qctr[0] += 1
        return q

    with tile.TileContext(nc) as tc:
        with tc.tile_pool(name="const", bufs=1) as cp, \
             tc.tile_pool(name="glo", bufs=4) as gplo, \
             tc.tile_pool(name="ghi", bufs=4) as gphi, \
             tc.tile_pool(name="smat", bufs=3) as sp_, \
             tc.tile_pool(name="sbwork", bufs=3) as wp, \
             tc.tile_pool(name="psum_a", bufs=2, space="PSUM") as pa, \
             tc.tile_pool(name="psum_h", bufs=2, space="PSUM") as ph, \
             tc.tile_pool(name="psum_p", bufs=1, space="PSUM") as pp:

            def load_const(t, shape, dtype=f32):
                tl = cp.tile(list(shape), dtype, tag=t.name)
                nc.sync.dma_start(out=tl[:], in_=t.ap())
                return tl

            eidx_t = load_const(eidx, [128, cfg.EIDX_COLS], i16)
            diag_t = load_const(diag, [128, cfg.NBLK * 128], bf16)
            po_t = load_const(po, [128, cfg.NBLK * cfg.NG], bf16)
            wts_t = [load_const(w, [D, D], bf16) for w in wts]
            w3_t = load_const(w3, [D, D])
            bbs_t = [load_const(b, [128, D]) for b in bbs]
            b3c_t = load_const(b3c, [128, 1])
            linw_t = load_const(linw, [D, cfg.NCLS])
            linb_t = load_const(linb, [cfg.NCLS, 1])

            pool_ps = pp.tile([128, cfg.NG], f32)

            def table_views(l):
                table = xp if l == 0 else h_ful[l - 1]
                return (table.ap()[0:cfg.LO_SIZE, :],
                        table.ap()[cfg.HI_BASE:cfg.NPAD, :])

            def issue_gathers(l, s, nbk, prep_sems=None):
                lo_view, hi_view = table_views(l)
                tglo = gplo.tile([128, cfg.TCOL_LO, 128], bf16, tag="tglo")
                tghi = gphi.tile([128, cfg.TCOL_HI, 128], bf16, tag="tghi")
                o0, o1 = cfg.eidx_off[s]
                n_lo = nbk * cfg.CHALF_LO
                n_hi = nbk * cfg.CHALF_HI
                q0, q1 = next_q(), next_q()
                kw0 = dict(prepare_only=True, sem=prep_sems[0]) if prep_sems else {}
                kw1 = dict(prepare_only=True, sem=prep_sems[1]) if prep_sems else {}
                nc.gpsimd.dma_gather(
                    tglo[:, 0:nbk * cfg.NG_LO, :], lo_view,
                    eidx_t[:, o0:o0 + n_lo // 16],
                    num_idxs=n_lo, num_idxs_reg=n_lo, elem_size=128,
                    single_packet=False, queue_num=q0, **kw0)
                nc.gpsimd.dma_gather(
                    tghi[:, 0:nbk * cfg.NG_HI, :], hi_view,
                    eidx_t[:, o1:o1 + n_hi // 16],
                    num_idxs=n_hi, num_idxs_reg=n_hi, elem_size=128,
                    single_packet=False, queue_num=q1, **kw1)
                return tglo, tghi, (q0, q1)

            N_PREP = 0
            pre_tiles = {}
            prep_queues = []

            for l in range(2):
                selftab = xself if l == 0 else h_loc[l - 1]
                for s, nbk in enumerate(cfg.SB):
                    bbase = s_block_base(cfg, s)
                    if (l, s) in pre_tiles:
                        tglo, tghi = pre_tiles.pop((l, s))
                    else:
                        tglo, tghi, _ = issue_gathers(l, s, nbk)
                    smat_t = sp_.tile([128, cfg.STILE], bf16, tag="smat_t")
                    nc.sync.dma_start(
                        out=smat_t[:, 0:nbk * cfg.NGRP * 128],
                        in_=smat.ap()[:, bbase * cfg.NGRP * 128:
                                      (bbase + nbk) * cfg.NGRP * 128])
                    for j in range(nbk):
                        blk = bbase + j
                        hb = wp.tile([128, 128], bf16, tag="hb")
                        aggT = pa.tile([128, 128], f32, tag="aggT")
                        for gg in range(cfg.NGRP):
                            if gg < cfg.NG_LO:
                                tcol = j * cfg.NG_LO + gg
                                src_sl = tglo[:, tcol, :]
                            else:
                                tcol = j * cfg.NG_HI + (gg - cfg.NG_LO)
                                src_sl = tghi[:, tcol, :]
                            scol = (j * cfg.NGRP + gg) * 128
                            nc.tensor.matmul(
                                aggT[:], lhsT=src_sl,
                                rhs=smat_t[:, scol:scol + 128],
                                start=(gg == 0), stop=False)
                        # self-loop: contiguous row read + diagonal matrix
                        nc.sync.dma_start(
                            out=hb[:],
                            in_=selftab.ap()[blk * 128:(blk + 1) * 128, :])
                        nc.tensor.matmul(
                            aggT[:], lhsT=hb[:],
                            rhs=diag_t[:, blk * 128:(blk + 1) * 128],
                            start=False, stop=True)
                        aggs = wp.tile([128, 128], bf16, tag="aggs")
                        nc.scalar.activation(aggs[:], aggT[:], AF.Copy)
                        hp = ph.tile([128, 128], f32, tag="hp")
                        nc.tensor.matmul(hp[:], lhsT=aggs[:], rhs=wts_t[l][:],
                                         start=True, stop=True)
                        hs = wp.tile([128, 128], bf16, tag="hs")
                        nc.vector.tensor_tensor(hs[:], hp[:], bbs_t[l][:],
                                                OP.add)
                        nc.vector.tensor_relu(hs[:], hs[:])
                        if l == 0:
                            nc.sync.dma_start(
                                out=h_loc[0].ap()[blk * 128:(blk + 1) * 128, :],
                                in_=hs[:])
                        else:
                            # fused layer3 aggregation + mean pool, by source
                            nc.tensor.matmul(
                                pool_ps[:], lhsT=hs[:],
                                rhs=po_t[:, blk * cfg.NG:(blk + 1) * cfg.NG],
                                start=(blk == 0), stop=(blk == cfg.NBLK - 1))
                if l == 0:
                    # prepare layer-2's first gathers now: descriptor
                    # generation (the serial Q7 bottleneck) overlaps the
                    # tail compute and the AllGather; triggers fire after.
                    for ps in range(N_PREP):
                        sems = (nc.alloc_semaphore(f"pgl{ps}"),
                                nc.alloc_semaphore(f"pgh{ps}"))
                        tg, th, qs = issue_gathers(1, ps, cfg.SB[ps],
                                                   prep_sems=sems)
                        pre_tiles[(1, ps)] = (tg, th)
                        prep_queues.extend(qs)
                    nc.gpsimd.collective_compute(
                        "AllGather", mybir.AluOpType.bypass,
                        replica_groups=groups,
                        ins=[h_loc[0].ap()], outs=[h_ful[0].ap()])
                    if prep_queues:
                        # Pool-engine read of h_ful0: Tile attaches the
                        # AllGather-completion wait here; the triggers that
                        # follow in Pool program order are therefore safe.
                        guard = wp.tile([1, 64], bf16, tag="agguard")
                        nc.gpsimd.dma_start(out=guard[:],
                                            in_=h_ful[0].ap()[0:1, 0:64])
                    for q in prep_queues:
                        nc.gpsimd.trigger_dma(count=None, queue_num=q)

            # epilogue: AllReduce pooled_agg3, then (pooled_agg3 @ W3 + b3) @ lin
            pool_sb = wp.tile([128, cfg.NG], f32, tag="pool_sb")
            nc.scalar.activation(pool_sb[:], pool_ps[:], AF.Copy)
            nc.sync.dma_start(out=pool_in.ap(), in_=pool_sb[:])
            nc.gpsimd.collective_compute(
                "AllReduce", mybir.AluOpType.add, replica_groups=groups,
                ins=[pool_in.ap()], outs=[pool_out.ap()])
            psum2 = wp.tile([128, cfg.NG], f32, tag="psum2")
            nc.sync.dma_start(out=psum2[:], in_=pool_out.ap())
            h3_ps = pp.tile([128, cfg.NG], f32, tag="h3_ps")
            nc.tensor.matmul(h3_ps[:], lhsT=w3_t[:], rhs=psum2[:],
                             start=True, stop=True)
            h3_sb = wp.tile([128, cfg.NG], f32, tag="h3_sb")
            nc.vector.tensor_scalar(h3_sb[:], h3_ps[:], b3c_t[:, 0:1], None,
                                    OP.add)
            out_ps = pp.tile([cfg.NCLS, cfg.NG], f32, tag="out_ps")
            nc.tensor.matmul(out_ps[:], lhsT=linw_t[:], rhs=h3_sb[:],
                             start=True, stop=True)
            outs = wp.tile([cfg.NCLS, cfg.NG], f32, tag="outs")
            nc.vector.tensor_scalar(outs[:], out_ps[:], linb_t[:, 0:1], None,
                                    OP.add)
            nc.sync.dma_start(out=out_t.ap(), in_=outs[:])

    nc.compile()
    return nc


def _get_program(cfg):
    key = (cfg.N, cfg.E, cfg.NG, cfg.NCLS, cfg.NBLK, cfg.CSLOT, cfg.NQ)
    if key not in _PROGRAM_CACHE:
        _PROGRAM_CACHE[key] = _build_program(cfg)
    return _PROGRAM_CACHE[key]


# ---------------------------------------------------------------------------
# entry point
# ---------------------------------------------------------------------------

def _run(cfg, x, edge_index, batch, W1, b1, W2, b2, W3, b3, lin_w, lin_b,
         trace=False):
    from concourse import bass_utils

    pre = _preprocess(cfg, x, edge_index, batch)
    nc = _get_program(cfg)

    shared = {
        "w0": np.asarray(W1, np.float32).astype(BF16),
        "w1": np.asarray(W2, np.float32).astype(BF16),
        "w3": np.asarray(W3, np.float32),
        "bb0": np.tile(np.asarray(b1, np.float32), (128, 1)),
        "bb1": np.tile(np.asarray(b2, np.float32), (128, 1)),
        "b3c": np.asarray(b3, np.float32).reshape(128, 1),
        "linw": np.asarray(lin_w, np.float32),
        "linb": np.asarray(lin_b, np.float32).reshape(cfg.NCLS, 1),
    }
    in_maps = []
    for c in range(cfg.NCORES):
        m = dict(shared)
        m["xp"] = pre["x_perm"]
        m["xself"] = pre["x_perm"][c * cfg.PER_CORE:(c + 1) * cfg.PER_CORE]
        m["eidx"] = pre["eidx"][c]
        m["smat"] = pre["smat"][c]
        m["diag"] = pre["diag"][c]
        m["po"] = pre["po"][c]
        in_maps.append(m)

    res = bass_utils.run_bass_kernel_spmd(
        nc, in_maps, core_ids=list(range(cfg.NCORES)), trace=trace)
    out = np.asarray(res.results[0]["out_t"]).T.copy()
    return out, res


def kernel(x, edge_index, batch, W1, b1, W2, b2, W3, b3, lin_w, lin_b):
    out, _ = _run(FULL, x, edge_index, batch, W1, b1, W2, b2, W3, b3,
                  lin_w, lin_b, trace=False)
    return out


# revision 23
# speedup vs baseline: 1.1813x; 1.0042x over previous
"""Trainium2 Bass kernel for a 3-layer GCN (PyG GCNConv x3 + global mean pool + linear).

Strategy (8 NeuronCores, SPMD), v2:
  - Nodes relabeled on the host (edge-balanced blocks of 128 via LPT) and
    partitioned contiguously across 8 cores; each core owns the edges whose
    destination lands in its partition.
  - All node-feature tables (x, h) are bf16: gathers move 256B rows, halving
    HBM traffic and SWDGE descriptor drain time.
  - The per-(block,group) scatter one-hot matrices (norm folded in) are
    precomputed on the host in bf16 and STREAMED from HBM via HWDGE — no
    on-device one-hot construction (the v1 VectorE bottleneck).
  - Self-loop diagonal matrices and mean-pool one-hots are layer-invariant
    bf16 constants resident in SBUF.
  - Per layer: SWDGE dma_gathers of 256B rows + scatter matmuls accumulating
    in PSUM (fp32); h' = relu(agg @ W + b) with bf16 weights; AllGather (bf16)
    shares node features; AllReduce at the mean pool; final linear on-device.
  - v3: layer 3 and the mean pool are linear, so they commute:
    pooled(agg3 @ W3 + b3) = pooled_agg3 @ W3 + b3, and pooled_agg3 regrouped
    BY SOURCE block needs no gather (h2 rows are core-local): it is one
    [128 x 64] matmul per block against a host-precomputed per-src-block
    weight matrix G (edge norms, self-loops and 1/graph-count folded in).
    This removes the third gather pass, its scatter-matrix stream, the second
    AllGather and the h2 store entirely.
"""

import numpy as np
import ml_dtypes

BF16 = ml_dtypes.bfloat16

# ---------------------------------------------------------------------------
# configuration
# ---------------------------------------------------------------------------

class CFG:
    def __init__(self, n_nodes, n_edges, n_graphs, n_cls, nblk, hi_base, lo_size,
                 chalf_lo=1024, chalf_hi=1024, super_blocks=4, nqueues=4):
        self.N = n_nodes
        self.E = n_edges
        self.NG = n_graphs
        self.NCLS = n_cls
        self.NBLK = nblk                       # blocks (of 128 dst nodes) per core
        self.NCORES = 8
        self.NQ = nqueues
        self.PER_CORE = nblk * 128
        self.NPAD = self.NCORES * self.PER_CORE
        self.LO_SIZE = lo_size                 # lo gather table = rows [0, LO_SIZE)
        self.HI_BASE = hi_base                 # hi gather table = rows [HI_BASE, NPAD)
        assert lo_size <= 32768 and (self.NPAD - hi_base) <= 32768
        self.CHALF_LO = chalf_lo               # lo edge slots per block
        self.CHALF_HI = chalf_hi
        assert chalf_lo % 128 == 0 and chalf_hi % 128 == 0
        self.NG_LO = chalf_lo // 128           # column-groups
        self.NG_HI = chalf_hi // 128
        self.NGRP = self.NG_LO + self.NG_HI    # edge groups per block
        self.CSLOT = chalf_lo + chalf_hi       # edge slots per block
        # super-blocks: group consecutive blocks into one gather call
        sb = []
        left = nblk
        while left > 0:
            take = min(super_blocks, left)
            sb.append(take)
            left -= take
        self.SB = sb
        self.SBMAX = max(sb)
        self.TCOL_LO = self.SBMAX * self.NG_LO     # tile cols of the lo tile
        self.TCOL_HI = self.SBMAX * self.NG_HI
        # scatter-matrix tensor: per block NGRP groups of 128 dst columns
        self.SCOLS = nblk * self.NGRP * 128
        self.STILE = self.SBMAX * self.NGRP * 128  # streamed slice per SB
        # eidx packing offsets (in int16 columns of the [128, *] index tile)
        self.eidx_off = []
        off = 0
        for nb in sb:
            lo_cols = nb * self.CHALF_LO // 16
            hi_cols = nb * self.CHALF_HI // 16
            self.eidx_off.append((off, off + lo_cols))
            off += lo_cols + hi_cols
        self.EIDX_COLS = off


FULL = CFG(n_nodes=50000, n_edges=800000, n_graphs=64, n_cls=16,
           nblk=49, hi_base=17408, lo_size=32768)


# ---------------------------------------------------------------------------
# host preprocessing
# ---------------------------------------------------------------------------

def _preprocess(cfg, x, edge_index, batch):
    """Relabel nodes, partition edges, build all device-side tables."""
    import heapq
    N = cfg.N
    src = np.asarray(edge_index[0], dtype=np.int64)
    dst = np.asarray(edge_index[1], dtype=np.int64)
    batch = np.asarray(batch, dtype=np.int64)

    deg = (np.bincount(dst, minlength=N) + 1).astype(np.float64)
    dinv = (1.0 / np.sqrt(deg)).astype(np.float32)
    dinv2 = (dinv * dinv).astype(np.float32)   # self-loop weight

    norm_all = (dinv[src] * dinv[dst]).astype(np.float32)

    # --- LPT assignment of nodes to blocks, balancing incoming edge counts ---
    w = np.bincount(dst, minlength=N)          # edge slots demanded per node
    nblocks = cfg.NCORES * cfg.NBLK
    order = np.argsort(-w, kind="stable")
    heap = [(0, 0, b) for b in range(nblocks)]
    heapq.heapify(heap)
    blk_of = np.empty(N, np.int64)
    pos_of = np.empty(N, np.int64)
    for v in order:
        while True:
            load, cnt, b = heapq.heappop(heap)
            if cnt < 128:
                break
        blk_of[v] = b
        pos_of[v] = cnt
        heapq.heappush(heap, (load + int(w[v]), cnt + 1, b))
    new_id = blk_of * 128 + pos_of                 # [N] -> [0, NPAD)

    x_perm = np.zeros((cfg.NPAD, x.shape[1]), dtype=np.float32)
    x_perm[new_id] = np.asarray(x, dtype=np.float32)
    x_perm = x_perm.astype(BF16)

    nsrc = new_id[src]
    ndst = new_id[dst]
    nb_of_e = ndst // 128
    eorder = np.argsort(nb_of_e, kind="stable")
    nsrc, ndst, norm_all, nb_of_e = (nsrc[eorder], ndst[eorder],
                                     norm_all[eorder], nb_of_e[eorder])
    blk_start = np.searchsorted(nb_of_e, np.arange(nblocks + 1))

    dinv2_new = np.zeros(cfg.NPAD, dtype=np.float32)
    dinv2_new[new_id] = dinv2

    # fused layer-3 + mean-pool weights, grouped by SOURCE node:
    # pooled_agg3[g] = sum_e (dinv_s*dinv_d/cnt_gd) h2[src_e] + sum_v dinv2_v/cnt_gv h2[v]
    counts = np.bincount(batch, minlength=cfg.NG).astype(np.float32)
    cnt = np.maximum(counts, 1.0)
    alpha = dinv[src] * dinv[dst] / cnt[batch[dst]]
    gall = np.bincount(new_id[src] * cfg.NG + batch[dst], weights=alpha,
                       minlength=cfg.NPAD * cfg.NG)
    gall += np.bincount(new_id * cfg.NG + batch, weights=dinv2 / cnt[batch],
                        minlength=cfg.NPAD * cfg.NG)
    gall = gall.reshape(cfg.NPAD, cfg.NG).astype(np.float32)

    pidx = np.arange(128)
    eidx_all, smat_all, diag_all, po_all = [], [], [], []
    for c in range(cfg.NCORES):
        eidx = np.zeros((16, cfg.EIDX_COLS), dtype=np.int16)
        smat = np.zeros((128, cfg.SCOLS), dtype=np.float32)
        for s, nbk in enumerate(cfg.SB):
            lo_idx_cat, hi_idx_cat = [], []
            for j in range(nbk):
                blk_local = s_block_base(cfg, s) + j
                b = c * cfg.NBLK + blk_local
                lo_e, hi_e = _split_block_edges(
                    cfg, nsrc[blk_start[b]:blk_start[b + 1]],
                    ndst[blk_start[b]:blk_start[b + 1]],
                    norm_all[blk_start[b]:blk_start[b + 1]])
                (li, ld, ln), (hi_, hd, hn) = lo_e, hi_e
                lo_idx_cat.append(li)
                hi_idx_cat.append(hi_)
                # scatter matrices with norm folded in (bf16 later)
                for goff, dvals, nvals in ((0, ld, ln), (cfg.NG_LO, hd, hn)):
                    k = np.flatnonzero(nvals != 0)
                    part = k % 128
                    grp = k // 128 + goff
                    col = (blk_local * cfg.NGRP + grp) * 128 + dvals[k].astype(np.int64)
                    smat[part, col] = nvals[k]
            lo_cat = np.concatenate(lo_idx_cat)
            hi_cat = np.concatenate(hi_idx_cat)
            o0, o1 = cfg.eidx_off[s]
            eidx[:, o0:o1] = lo_cat.reshape(-1, 16).T
            eidx[:, o1:o1 + hi_cat.size // 16] = hi_cat.reshape(-1, 16).T
        base = c * cfg.PER_CORE
        # self-loop diagonal matrices [128, NBLK*128]
        diag = np.zeros((128, cfg.NBLK * 128), dtype=np.float32)
        dv = dinv2_new[base:base + cfg.PER_CORE].reshape(cfg.NBLK, 128)
        for blk in range(cfg.NBLK):
            diag[pidx, blk * 128 + pidx] = dv[blk]
        # fused layer3+pool weights [128, NBLK*NG] (src-block-major)
        po = (gall[base:base + cfg.PER_CORE]
              .reshape(cfg.NBLK, 128, cfg.NG)
              .transpose(1, 0, 2)
              .reshape(128, cfg.NBLK * cfg.NG))
        eidx_all.append(np.tile(eidx, (8, 1)))
        smat_all.append(smat.astype(BF16))
        diag_all.append(diag.astype(BF16))
        po_all.append(po.astype(BF16).copy())

    return dict(x_perm=x_perm, eidx=eidx_all, smat=smat_all, diag=diag_all,
                po=po_all)


def s_block_base(cfg, s):
    return sum(cfg.SB[:s])


def _split_block_edges(cfg, nsrc_b, ndst_b, nrm_b):
    """Split one block's edges into lo/hi gather halves and pad."""
    CH_LO, CH_HI = cfg.CHALF_LO, cfg.CHALF_HI
    n = nsrc_b.size
    assert n <= CH_LO + CH_HI, f"block overflow: {n} > {CH_LO + CH_HI}"
    strict_lo = nsrc_b < cfg.HI_BASE
    strict_hi = nsrc_b >= cfg.LO_SIZE
    mid = ~strict_lo & ~strict_hi
    n_strict_lo = int(strict_lo.sum())
    n_strict_hi = int(strict_hi.sum())
    assert n_strict_lo <= CH_LO, "lo half overflow"
    assert n_strict_hi <= CH_HI, "hi half overflow"
    take_mid_lo = min(int(mid.sum()), CH_LO - n_strict_lo)
    mid_idx = np.flatnonzero(mid)
    lo_sel = np.concatenate([np.flatnonzero(strict_lo), mid_idx[:take_mid_lo]])
    hi_sel = np.concatenate([np.flatnonzero(strict_hi), mid_idx[take_mid_lo:]])
    lo_sel = lo_sel[np.argsort(nsrc_b[lo_sel], kind="stable")]
    hi_sel = hi_sel[np.argsort(nsrc_b[hi_sel], kind="stable")]
    assert hi_sel.size <= CH_HI, "hi half overflow after balance"
    dl = (ndst_b % 128).astype(np.int64)

    def pack(sel, base, cap):
        idx = np.zeros(cap, np.int16)
        d = np.zeros(cap, np.int64)
        nm = np.zeros(cap, np.float32)
        k = sel.size
        idx[:k] = (nsrc_b[sel] - base).astype(np.int16)
        d[:k] = dl[sel]
        nm[:k] = nrm_b[sel]
        return idx, d, nm

    return pack(lo_sel, 0, CH_LO), pack(hi_sel, cfg.HI_BASE, CH_HI)


# ---------------------------------------------------------------------------
# device program
# ---------------------------------------------------------------------------

_PROGRAM_CACHE = {}


def _build_program(cfg):
    import concourse.bacc as bacc
    import concourse.tile as tile
    import concourse.mybir as mybir

    f32 = mybir.dt.float32
    bf16 = mybir.dt.bfloat16
    i16 = mybir.dt.int16
    AF = mybir.ActivationFunctionType
    OP = mybir.AluOpType

    nc = bacc.Bacc("TRN2", target_bir_lowering=False, debug=False,
                   num_devices=cfg.NCORES, num_swdge_queues=cfg.NQ,
                   dynamic_dma_scratch_size=24576)

    D = 128
    xp = nc.dram_tensor("xp", [cfg.NPAD, D], bf16, kind="ExternalInput")
    xself = nc.dram_tensor("xself", [cfg.PER_CORE, D], bf16, kind="ExternalInput")
    eidx = nc.dram_tensor("eidx", [128, cfg.EIDX_COLS], i16, kind="ExternalInput")
    smat = nc.dram_tensor("smat", [128, cfg.SCOLS], bf16, kind="ExternalInput")
    diag = nc.dram_tensor("diag", [128, cfg.NBLK * 128], bf16, kind="ExternalInput")
    po = nc.dram_tensor("po", [128, cfg.NBLK * cfg.NG], bf16, kind="ExternalInput")
    wts = [nc.dram_tensor(f"w{l}", [D, D], bf16, kind="ExternalInput") for l in range(2)]
    w3 = nc.dram_tensor("w3", [D, D], f32, kind="ExternalInput")
    bbs = [nc.dram_tensor(f"bb{l}", [128, D], f32, kind="ExternalInput") for l in range(2)]
    b3c = nc.dram_tensor("b3c", [128, 1], f32, kind="ExternalInput")
    linw = nc.dram_tensor("linw", [D, cfg.NCLS], f32, kind="ExternalInput")
    linb = nc.dram_tensor("linb", [cfg.NCLS, 1], f32, kind="ExternalInput")

    out_t = nc.dram_tensor("out_t", [cfg.NCLS, cfg.NG], f32, kind="ExternalOutput")

    h_loc = [nc.dram_tensor("h_loc0", [cfg.PER_CORE, D], bf16, kind="Internal")]
    h_ful = [nc.dram_tensor("h_ful0", [cfg.NPAD, D], bf16, kind="Internal",
                            addr_space="Shared")]
    pool_in = nc.dram_tensor("pool_in", [128, cfg.NG], f32, kind="Internal")
    pool_out = nc.dram_tensor("pool_out", [128, cfg.NG], f32, kind="Internal",
                              addr_space="Shared")

    groups = [list(range(cfg.NCORES))]
    qctr = [0]

    def next_q():
        q = qctr[0] % cfg.NQ
        qctr[0] += 1
        return q

    with tile.TileContext(nc) as tc:
        with tc.tile_pool(name="const", bufs=1) as cp, \
             tc.tile_pool(name="glo", bufs=4) as gplo, \
             tc.tile_pool(name="ghi", bufs=4) as gphi, \
             tc.tile_pool(name="smat", bufs=3) as sp_, \
             tc.tile_pool(name="sbwork", bufs=3) as wp, \
             tc.tile_pool(name="psum_a", bufs=2, space="PSUM") as pa, \
             tc.tile_pool(name="psum_h", bufs=2, space="PSUM") as ph, \
             tc.tile_pool(name="psum_p", bufs=1, space="PSUM") as pp:

            def load_const(t, shape, dtype=f32):
                tl = cp.tile(list(shape), dtype, tag=t.name)
                nc.sync.dma_start(out=tl[:], in_=t.ap())
                return tl

            eidx_t = load_const(eidx, [128, cfg.EIDX_COLS], i16)
            diag_t = load_const(diag, [128, cfg.NBLK * 128], bf16)
            po_t = load_const(po, [128, cfg.NBLK * cfg.NG], bf16)
            wts_t = [load_const(w, [D, D], bf16) for w in wts]
            w3_t = load_const(w3, [D, D])
            bbs_t = [load_const(b, [128, D]) for b in bbs]
            b3c_t = load_const(b3c, [128, 1])
            linw_t = load_const(linw, [D, cfg.NCLS])
            linb_t = load_const(linb, [cfg.NCLS, 1])

            pool_ps = pp.tile([128, cfg.NG], f32)

            def table_views(l):
                table = xp if l == 0 else h_ful[l - 1]
                return (table.ap()[0:cfg.LO_SIZE, :],
                        table.ap()[cfg.HI_BASE:cfg.NPAD, :])

            def issue_gathers(l, s, nbk, prep_sems=None):
                lo_view, hi_view = table_views(l)
                tglo = gplo.tile([128, cfg.TCOL_LO, 128], bf16, tag="tglo")
                tghi = gphi.tile([128, cfg.TCOL_HI, 128], bf16, tag="tghi")
                o0, o1 = cfg.eidx_off[s]
                n_lo = nbk * cfg.CHALF_LO
                n_hi = nbk * cfg.CHALF_HI
                q0, q1 = next_q(), next_q()
                kw0 = dict(prepare_only=True, sem=prep_sems[0]) if prep_sems else {}
                kw1 = dict(prepare_only=True, sem=prep_sems[1]) if prep_sems else {}
                nc.gpsimd.dma_gather(
                    tglo[:, 0:nbk * cfg.NG_LO, :], lo_view,
                    eidx_t[:, o0:o0 + n_lo // 16],
                    num_idxs=n_lo, num_idxs_reg=n_lo, elem_size=128,
                    single_packet=False, queue_num=q0, **kw0)
                nc.gpsimd.dma_gather(
                    tghi[:, 0:nbk * cfg.NG_HI, :], hi_view,
                    eidx_t[:, o1:o1 + n_hi // 16],
                    num_idxs=n_hi, num_idxs_reg=n_hi, elem_size=128,
                    single_packet=False, queue_num=q1, **kw1)
                return tglo, tghi, (q0, q1)

            N_PREP = 0
            pre_tiles = {}
            prep_queues = []

            for l in range(2):
                selftab = xself if l == 0 else h_loc[l - 1]
                for s, nbk in enumerate(cfg.SB):
                    bbase = s_block_base(cfg, s)
                    if (l, s) in pre_tiles:
                        tglo, tghi = pre_tiles.pop((l, s))
                    else:
                        tglo, tghi, _ = issue_gathers(l, s, nbk)
                    smat_t = sp_.tile([128, cfg.STILE], bf16, tag="smat_t")
                    nc.sync.dma_start(
                        out=smat_t[:, 0:nbk * cfg.NGRP * 128],
                        in_=smat.ap()[:, bbase * cfg.NGRP * 128:
                                      (bbase + nbk) * cfg.NGRP * 128])
                    for j in range(nbk):
                        blk = bbase + j
                        hb = wp.tile([128, 128], bf16, tag="hb")
                        aggT = pa.tile([128, 128], f32, tag="aggT")
                        for gg in range(cfg.NGRP):
                            if gg < cfg.NG_LO:
                                tcol = j * cfg.NG_LO + gg
                                src_sl = tglo[:, tcol, :]
                            else:
                                tcol = j * cfg.NG_HI + (gg - cfg.NG_LO)
                                src_sl = tghi[:, tcol, :]
                            scol = (j * cfg.NGRP + gg) * 128
                            nc.tensor.matmul(
                                aggT[:], lhsT=src_sl,
                                rhs=smat_t[:, scol:scol + 128],
                                start=(gg == 0), stop=False)
                        # self-loop: contiguous row read + diagonal matrix
                        nc.sync.dma_start(
                            out=hb[:],
                            in_=selftab.ap()[blk * 128:(blk + 1) * 128, :])
                        nc.tensor.matmul(
                            aggT[:], lhsT=hb[:],
                            rhs=diag_t[:, blk * 128:(blk + 1) * 128],
                            start=False, stop=True)
                        aggs = wp.tile([128, 128], bf16, tag="aggs")
                        nc.scalar.activation(aggs[:], aggT[:], AF.Copy)
                        hp = ph.tile([128, 128], f32, tag="hp")
                        nc.tensor.matmul(hp[:], lhsT=aggs[:], rhs=wts_t[l][:],
                                         start=True, stop=True)
                        hs = wp.tile([128, 128], bf16, tag="hs")
                        nc.vector.tensor_tensor(hs[:], hp[:], bbs_t[l][:],
                                                OP.add)
                        nc.vector.tensor_relu(hs[:], hs[:])
                        if l == 0:
                            nc.sync.dma_start(
                                out=h_loc[0].ap()[blk * 128:(blk + 1) * 128, :],
                                in_=hs[:])
                        else:
                            # fused layer3 aggregation + mean pool, by source
                            nc.tensor.matmul(
                                pool_ps[:], lhsT=hs[:],
                                rhs=po_t[:, blk * cfg.NG:(blk + 1) * cfg.NG],
                                start=(blk == 0), stop=(blk == cfg.NBLK - 1))
                if l == 0:
                    # prepare layer-2's first gathers now: descriptor
                    # generation (the serial Q7 bottleneck) overlaps the
                    # tail compute and the AllGather; triggers fire after.
                    for ps in range(N_PREP):
                        sems = (nc.alloc_semaphore(f"pgl{ps}"),
                                nc.alloc_semaphore(f"pgh{ps}"))
                        tg, th, qs = issue_gathers(1, ps, cfg.SB[ps],
                                                   prep_sems=sems)
                        pre_tiles[(1, ps)] = (tg, th)
                        prep_queues.extend(qs)
                    nc.gpsimd.collective_compute(
                        "AllGather", mybir.AluOpType.bypass,
                        replica_groups=groups,
                        ins=[h_loc[0].ap()], outs=[h_ful[0].ap()])
                    if prep_queues:
                        # Pool-engine read of h_ful0: Tile attaches the
                        # AllGather-completion wait here; the triggers that
                        # follow in Pool program order are therefore safe.
                        guard = wp.tile([1, 64], bf16, tag="agguard")
                        nc.gpsimd.dma_start(out=guard[:],
                                            in_=h_ful[0].ap()[0:1, 0:64])
                    for q in prep_queues:
                        nc.gpsimd.trigger_dma(count=None, queue_num=q)

            # epilogue: AllReduce pooled_agg3, then (pooled_agg3 @ W3 + b3) @ lin
            pool_sb = wp.tile([128, cfg.NG], f32, tag="pool_sb")
            nc.scalar.activation(pool_sb[:], pool_ps[:], AF.Copy)
            nc.sync.dma_start(out=pool_in.ap(), in_=pool_sb[:])
            nc.gpsimd.collective_compute(
                "AllReduce", mybir.AluOpType.add, replica_groups=groups,
                ins=[pool_in.ap()], outs=[pool_out.ap()])
            psum2 = wp.tile([128, cfg.NG], f32, tag="psum2")
            nc.sync.dma_start(out=psum2[:], in_=pool_out.ap())
            h3_ps = pp.tile([128, cfg.NG], f32, tag="h3_ps")
            nc.tensor.matmul(h3_ps[:], lhsT=w3_t[:], rhs=psum2[:],
                             start=True, stop=True)
            h3_sb = wp.tile([128, cfg.NG], f32, tag="h3_sb")
            nc.vector.tensor_scalar(h3_sb[:], h3_ps[:], b3c_t[:, 0:1], None,
                                    OP.add)
            out_ps = pp.tile([cfg.NCLS, cfg.NG], f32, tag="out_ps")
            nc.tensor.matmul(out_ps[:], lhsT=linw_t[:], rhs=h3_sb[:],
                             start=True, stop=True)
            outs = wp.tile([cfg.NCLS, cfg.NG], f32, tag="outs")
            nc.vector.tensor_scalar(outs[:], out_ps[:], linb_t[:, 0:1], None,
                                    OP.add)
            nc.sync.dma_start(out=out_t.ap(), in_=outs[:])

    nc.compile()
    return nc


def _get_program(cfg):
    key = (cfg.N, cfg.E, cfg.NG, cfg.NCLS, cfg.NBLK, cfg.CSLOT, cfg.NQ)
    if key not in _PROGRAM_CACHE:
        _PROGRAM_CACHE[key] = _build_program(cfg)
    return _PROGRAM_CACHE[key]


# ---------------------------------------------------------------------------
# entry point
# ---------------------------------------------------------------------------

def _run(cfg, x, edge_index, batch, W1, b1, W2, b2, W3, b3, lin_w, lin_b,
         trace=False):
    from concourse import bass_utils

    pre = _preprocess(cfg, x, edge_index, batch)
    nc = _get_program(cfg)

    shared = {
        "w0": np.asarray(W1, np.float32).astype(BF16),
        "w1": np.asarray(W2, np.float32).astype(BF16),
        "w3": np.asarray(W3, np.float32),
        "bb0": np.tile(np.asarray(b1, np.float32), (128, 1)),
        "bb1": np.tile(np.asarray(b2, np.float32), (128, 1)),
        "b3c": np.asarray(b3, np.float32).reshape(128, 1),
        "linw": np.asarray(lin_w, np.float32),
        "linb": np.asarray(lin_b, np.float32).reshape(cfg.NCLS, 1),
    }
    in_maps = []
    for c in range(cfg.NCORES):
        m = dict(shared)
        m["xp"] = pre["x_perm"]
        m["xself"] = pre["x_perm"][c * cfg.PER_CORE:(c + 1) * cfg.PER_CORE]
        m["eidx"] = pre["eidx"][c]
        m["smat"] = pre["smat"][c]
        m["diag"] = pre["diag"][c]
        m["po"] = pre["po"][c]
        in_maps.append(m)

    res = bass_utils.run_bass_kernel_spmd(
        nc, in_maps, core_ids=list(range(cfg.NCORES)), trace=trace)
    out = np.asarray(res.results[0]["out_t"]).T.copy()
    return out, res


def kernel(x, edge_index, batch, W1, b1, W2, b2, W3, b3, lin_w, lin_b):
    out, _ = _run(FULL, x, edge_index, batch, W1, b1, W2, b2, W3, b3,
                  lin_w, lin_b, trace=False)
    return out
